# revision 12
# baseline (speedup 1.0000x reference)
"""Erosion (5x5 sliding-window min, geodesic border pad 1e4) on TRN2.

Layout: partition p holds rows 8p..8p+7 of one image in segs 2..9, via
ONE SWDGE cast-DMA per image (f32->bf16) whose per-partition 32KB is
split into four 8KB-run descriptors (measured faster than one 32KB
run: more SDMA-engine interleave): 16 MiB HBM reads/core, no halo
re-read amplification. Halo rows (segs 0,1 = prev partition's segs 8,9; segs
10,11 = next partition's segs 2,3) are produced by the OTHERWISE-IDLE
PE + ACT engines: a shifted-identity bf16 matmul (exact 0/1 weights)
moves segs across partitions into PSUM, a second accumulated K=1
matmul writes the geodesic 1e4 pad into partition 0 (down-shift) /
127 (up-shift), and ACT copies PSUM back to the x-buffer with the
f32->bf16 cast. This costs ZERO DMA-ring time -- SBUF->SBUF SWDGE
measures only ~80 GB/s here (13.7 us/image for the halos, the reason
a DMA-based halo exchange loses), and HWDGE descriptor generation
chokes on 4KB-run layouts.

Compute is all-DVE full-width bf16 tensor_tensor min (2 elem/cycle
2x_1P mode, ~0.47 ns/elem measured): vertical w2/w4/v cascade, then a
3-pass horizontal cascade over column-padded tiles (v/a/b are W+4
wide with 1e4 in cols {0,1,W+2,W+3}, memset once; the vertical pass
writes only cols 2..W+2 so the pads survive, and a/b recompute their
own pad cols from v's pads inside the wide passes) -- no per-image
edge-column fixups. Stores cast bf16->f32 in the DMA (SWDGE, 16 MiB
writes/core).

Pipelining: the gpsimd queue per image is [loadA(k+3), store(k, segs
0..3), loadB(k+3), store(k, segs 4..7)]: the two load halves bracket
the DVE-gated store triggers so the FIFO ring always has ready load
work during both trigger waits; 4 x-buffers. Steady state is DMA-bound at the per-core HBM
share (~450 GB/s measured: 8.38 MB/image = ~18.6 us vs ~21 us DVE
which overlaps under it; doubling store traffic raises time ~425
GB/s-proportionally, while dropping a DVE pass changes almost
nothing). PE/ACT produce image k+1's halos while DVE computes image
k. The last image's second-half stores split in two for a shorter
drain. bf16 keeps rel err ~2e-3 (tolerance 2e-2).
"""

import numpy as np

import concourse.bacc as bacc
import concourse.mybir as mybir
import concourse.tile as tile
from concourse.bass import AP
from concourse.bass_utils import run_bass_kernel_spmd

B, H, W = 32, 1024, 1024
N_CORES = 8
PER_CORE = B // N_CORES     # 4 images per core
PX = 2
WP = W + 2 * PX             # padded row width for v/a/b tiles
PAD_VAL = 1e4
F32 = mybir.dt.float32
BF16 = mybir.dt.bfloat16
I32 = mybir.dt.int32
MIN = mybir.AluOpType.min
EQ = mybir.AluOpType.is_equal

KR = 8                      # output rows per partition (128*8 = 1024)
SEGS = KR + 2 * PX          # 12 segments per partition
MM = 512                    # max moving free dim per matmul

_CACHE = {}


def build_nc(repeat: int = 1):
    nc = bacc.Bacc("TRN2", debug=False, num_devices=N_CORES)
    x = nc.dram_tensor("mask", [PER_CORE, H, W], F32, kind="ExternalInput").ap()
    y = nc.dram_tensor("out", [PER_CORE, H, W], F32, kind="ExternalOutput").ap()

    N = repeat * PER_CORE   # flat image stream

    with tile.TileContext(nc) as tc:
        with (
            tc.tile_pool(name="const", bufs=1) as cpool,
            tc.tile_pool(name="xp", bufs=1) as xpool,
            tc.tile_pool(name="wp", bufs=1) as wpool,
            tc.tile_pool(name="op", bufs=1) as opool,
            tc.psum_pool(name="ps", bufs=1) as pspool,
        ):
            # 1e4 row for the geodesic pads (fed into PSUM by matmul)
            cpad = cpool.tile([128, PX * W], BF16)
            nc.vector.memset(cpad[:, :], PAD_VAL)

            # shifted-identity stationaries: idx[k, m] = m - k
            idx = cpool.tile([128, 128], I32)
            sdn = cpool.tile([128, 128], BF16)   # 1{k == m-1}: out[m]=x[m-1]
            sup = cpool.tile([128, 128], BF16)   # 1{k == m+1}: out[m]=x[m+1]
            e0 = cpool.tile([128, 128], BF16)    # row k=0: 1{m == 0}
            e127 = cpool.tile([128, 128], BF16)  # row k=0: 1{m == 127}
            nc.gpsimd.iota(idx[:, :], pattern=[[1, 128]], base=0,
                           channel_multiplier=-1)
            nc.vector.tensor_scalar(out=sdn[:, :], in0=idx[:, :],
                                    scalar1=1, scalar2=None, op0=EQ)
            nc.vector.tensor_scalar(out=sup[:, :], in0=idx[:, :],
                                    scalar1=-1, scalar2=None, op0=EQ)
            nc.vector.tensor_scalar(out=e0[:, :], in0=idx[:, :],
                                    scalar1=0, scalar2=None, op0=EQ)
            nc.vector.tensor_scalar(out=e127[:, :], in0=idx[:, :],
                                    scalar1=127, scalar2=None, op0=EQ)

            xbufs, obufs = [], []
            for i in range(4):
                xbufs.append(
                    xpool.tile([128, SEGS * W], BF16, tag=f"x{i}", name=f"xb{i}")
                )
            for i in range(2):
                obufs.append(
                    opool.tile([128, KR * W], BF16, tag=f"o{i}", name=f"ob{i}")
                )
            w2 = wpool.tile([128, (SEGS - 2) * W], BF16, tag="w2")
            w2_3 = w2[:, :].rearrange("p (s c) -> p s c", s=SEGS - 2)
            v = wpool.tile([128, KR * WP], BF16, tag="v")
            v3 = v[:, :].rearrange("p (s c) -> p s c", s=KR)
            aa = wpool.tile([128, KR * WP], BF16, tag="a")
            a3 = aa[:, :].rearrange("p (s c) -> p s c", s=KR)
            bb = wpool.tile([128, KR * WP], BF16, tag="b")
            b3 = bb[:, :].rearrange("p (s c) -> p s c", s=KR)
            psdn = pspool.tile([128, PX * W], F32, tag="psdn")
            psup = pspool.tile([128, PX * W], F32, tag="psup")

            # v's geodesic column pads, written once (vertical writes
            # only cols 2..W+2; a/b recompute theirs from v's pads)
            nc.gpsimd.memset(v3[:, :, 0:PX], PAD_VAL)
            nc.gpsimd.memset(v3[:, :, W + PX : WP], PAD_VAL)

            def issue_load_half(k, h):
                """SWDGE cast load (f32->bf16): 4 rows -> segs 2..5
                (h=0) or 6..9 (h=1), as 2x8KB runs per partition. Two
                halves per image, issued around store g1 so the ring
                always has ready load work during store-trigger waits."""
                img = k % PER_CORE
                xb = xbufs[k % 4]
                off = h * 4 * W
                nc.gpsimd.dma_start(
                    out=xb[0:128, PX * W + off : PX * W + off + 4 * W],
                    in_=AP(
                        x.tensor, img * H * W + off,
                        [[KR * W, 128], [2 * W, 2], [1, 2 * W]]
                    ),
                )

            def issue_halos(k):
                """PE shifted-identity matmuls + ACT PSUM->SBUF copies
                fill segs 0,1 (= prev partition's segs 8,9; 1e4 pad at
                p0) and segs 10,11 (= next partition's segs 2,3; pad at
                p127). No DMA-ring time at all."""
                xb = xbufs[k % 4]
                for j in range(0, PX * W, MM):
                    nc.tensor.matmul(
                        out=psdn[:, j : j + MM], lhsT=sdn[:, :],
                        rhs=xb[:, KR * W + j : KR * W + j + MM],
                        start=True, stop=False,
                    )
                    nc.tensor.matmul(
                        out=psdn[:, j : j + MM], lhsT=e0[0:1, :],
                        rhs=cpad[0:1, j : j + MM], start=False, stop=True,
                    )
                nc.scalar.copy(out=xb[0:128, 0 : PX * W], in_=psdn[:, :])
                for j in range(0, PX * W, MM):
                    nc.tensor.matmul(
                        out=psup[:, j : j + MM], lhsT=sup[:, :],
                        rhs=xb[:, PX * W + j : PX * W + j + MM],
                        start=True, stop=False,
                    )
                    nc.tensor.matmul(
                        out=psup[:, j : j + MM], lhsT=e127[0:1, :],
                        rhs=cpad[0:1, j : j + MM], start=False, stop=True,
                    )
                nc.scalar.copy(
                    out=xb[0:128, (PX + KR) * W : SEGS * W], in_=psup[:, :]
                )

            def horizontal(o3, sl):
                """3-pass a/b/o cascade over padded-width tiles."""
                nc.vector.tensor_tensor(
                    out=a3[:, sl, 0 : WP - 1],
                    in0=v3[:, sl, 0 : WP - 1],
                    in1=v3[:, sl, 1:WP],
                    op=MIN,
                )
                nc.vector.tensor_tensor(
                    out=b3[:, sl, 0 : W + 1],
                    in0=a3[:, sl, 0 : W + 1],
                    in1=a3[:, sl, 2 : W + 3],
                    op=MIN,
                )
                nc.vector.tensor_tensor(
                    out=o3[:, sl, 0:W],
                    in0=b3[:, sl, 0:W],
                    in1=v3[:, sl, 2 * PX : WP],
                    op=MIN,
                )

            def store(k, s0, nseg):
                """SWDGE cast store (bf16->f32): out segs s0..s0+nseg-1,
                partition p -> rows 8p+s0 .. (contiguous descriptors)."""
                img = k % PER_CORE
                ob = obufs[k % 2]
                nc.gpsimd.dma_start(
                    out=AP(
                        y.tensor,
                        img * H * W + s0 * W,
                        [[KR * W, 128], [1, nseg * W]],
                    ),
                    in_=ob[:, s0 * W : (s0 + nseg) * W],
                )

            # prologue
            issue_load_half(0, 0)
            issue_load_half(0, 1)
            issue_halos(0)
            if N > 1:
                issue_load_half(1, 0)
                issue_load_half(1, 1)
            if N > 2:
                issue_load_half(2, 0)
                issue_load_half(2, 1)

            for k in range(N):
                xb = xbufs[k % 4]
                ob = obufs[k % 2]
                x3 = xb[:, :].rearrange("p (s c) -> p s c", s=SEGS)
                o3 = ob[:, :].rearrange("p (s c) -> p s c", s=KR)

                # keep the SDMA ring fed, then let PE/ACT prepare the
                # NEXT image's halos while DVE computes this one
                if k + 3 < N:
                    issue_load_half(k + 3, 0)
                if k + 1 < N:
                    issue_halos(k + 1)

                # ---- group 1: out segs 0..3 (needs x segs 0..7) ----
                nc.vector.tensor_tensor(
                    out=w2_3[:, 0:7, :], in0=x3[:, 0:7, :], in1=x3[:, 1:8, :],
                    op=MIN,
                )
                # w4[0..4] in place (one extra for group 2's v[4])
                nc.vector.tensor_tensor(
                    out=w2_3[:, 0:5, :], in0=w2_3[:, 0:5, :], in1=w2_3[:, 2:7, :],
                    op=MIN,
                )
                nc.vector.tensor_tensor(
                    out=v3[:, 0:4, PX : W + PX],
                    in0=w2_3[:, 0:4, :],
                    in1=x3[:, 4:8, :],
                    op=MIN,
                )
                horizontal(o3, slice(0, 4))
                store(k, 0, 4)
                if k + 3 < N:
                    issue_load_half(k + 3, 1)

                # ---- group 2: out segs 4..7 (adds x segs 8..11) ----
                nc.vector.tensor_tensor(
                    out=w2_3[:, 7:10, :], in0=x3[:, 7:10, :], in1=x3[:, 8:11, :],
                    op=MIN,
                )
                nc.vector.tensor_tensor(
                    out=w2_3[:, 5:8, :], in0=w2_3[:, 5:8, :], in1=w2_3[:, 7:10, :],
                    op=MIN,
                )
                nc.vector.tensor_tensor(
                    out=v3[:, 4:8, PX : W + PX],
                    in0=w2_3[:, 4:8, :],
                    in1=x3[:, 8:12, :],
                    op=MIN,
                )
                if k == N - 1:
                    # shorter drain: two 2-seg pieces
                    horizontal(o3, slice(4, 6))
                    store(k, 4, 2)
                    horizontal(o3, slice(6, 8))
                    store(k, 6, 2)
                else:
                    horizontal(o3, slice(4, 8))
                    store(k, 4, 4)

    nc.compile()
    return nc


def run(mask: np.ndarray, trace: bool = False, tmpdir: str | None = None):
    assert mask.shape == (B, 1, H, W), mask.shape
    in_dtype = mask.dtype
    mask4 = np.ascontiguousarray(
        mask.reshape(B, H, W).astype(np.float32, copy=False)
    )
    if "nc" not in _CACHE:
        _CACHE["nc"] = build_nc(1)
    nc = _CACHE["nc"]
    in_maps = [
        {"mask": mask4[i * PER_CORE : (i + 1) * PER_CORE]} for i in range(N_CORES)
    ]
    res = run_bass_kernel_spmd(
        nc, in_maps, list(range(N_CORES)), trace=trace, tmpdir=tmpdir
    )
    out = np.concatenate([res.results[i]["out"] for i in range(N_CORES)], axis=0)
    return out.reshape(B, 1, H, W).astype(in_dtype, copy=False), res


def kernel(mask: np.ndarray) -> np.ndarray:
    return run(mask)[0]
